# revision 1
# baseline (speedup 1.0000x reference)
"""Bilinear sampler (spatial transformer) TRN2 Bass kernel.

Contract: kernel(inputs=[128, 196614] fp32) -> [128, 256, 256, 3] fp32.
Shards batch over 8 NeuronCores (16 images each). Per image on-device:
  - compute affine grid X = t00*j + t01*i + cx, Y likewise (ACT/DVE)
  - floors, bilinear weights with out-of-bounds masking (DVE)
  - build a row-pair interleaved copy of the image in DRAM scratch
    (site l = y*256+x holds rows y and y+1 of column x: 6 floats), so one
    contiguous 12-float fetch at offset 6*l yields the whole 2x2x3 patch
  - per pixel-column instruction: [P,1] indirect DMA gather (128 patches)
  - weighted blend of the 4 corners (DVE), DMA out
"""
import os
import sys

sys.path.insert(0, "/opt/trn_rl_repo")

import numpy as np

import concourse.bacc as bacc
import concourse.bass as bass
import concourse.mybir as mybir
import concourse.tile as tile
from concourse.bass_utils import run_bass_kernel_spmd

P = 128
H = W = 256
C = 3
IMG_ELS = H * W * C            # 196608
ROW_ELS = W * C                # 768
PW = (H * W) // P              # 512 pixels per partition per image
N_CORES = 8
IMGS = 16                      # images per core

F32 = mybir.dt.float32
I32 = mybir.dt.int32
ALU = mybir.AluOpType

_cached = {}


def _build(n_imgs):
    nc = bacc.Bacc("TRN2", target_bir_lowering=False, debug=False,
                   enable_asserts=False, num_devices=1, num_swdge_queues=2)
    inp = nc.dram_tensor("inp", [n_imgs, 6 + IMG_ELS], F32, kind="ExternalInput")
    xg_d = nc.dram_tensor("xg", [P, PW], F32, kind="ExternalInput")
    yg_d = nc.dram_tensor("yg", [P, PW], F32, kind="ExternalInput")
    cst_d = nc.dram_tensor("cst", [2, 4], F32, kind="ExternalInput")
    out_d = nc.dram_tensor("out", [n_imgs, H * W * C], F32, kind="ExternalOutput")
    idups = [nc.dram_tensor(f"idup{b}", [H * W, 6], F32) for b in range(n_imgs)]
    scr = nc.dram_tensor("scr", [n_imgs, 8], F32)

    with tile.TileContext(nc) as tc:
        with (
            tc.tile_pool(name="const", bufs=1) as cpool,
            tc.tile_pool(name="work", bufs=1) as wp,
            tc.tile_pool(name="gath", bufs=2) as gpool,
            tc.tile_pool(name="offp", bufs=2) as opool,
        ):
            xg = cpool.tile([P, PW], F32)
            nc.sync.dma_start(xg[:], xg_d[:, :])
            yg = cpool.tile([P, PW], F32)
            nc.sync.dma_start(yg[:], yg_d[:, :])
            cst = cpool.tile([2, 4], F32)
            nc.sync.dma_start(cst[:], cst_d[:, :])

            for b in range(n_imgs):
                # ---- affine params: [2,3] theta rows; cx/cy = 127.5*(t2+1-t0-t1)
                th = wp.tile([2, 3], F32)
                nc.sync.dma_start(th[:], bass.AP(inp, b * (6 + IMG_ELS), [[3, 2], [1, 3]]))
                m = wp.tile([2, 3], F32)
                nc.vector.tensor_tensor(out=m[:], in0=th[:], in1=cst[:, 0:3], op=ALU.mult)
                s = wp.tile([2, 1], F32)
                nc.vector.tensor_reduce(out=s[:], in_=m[:], axis=mybir.AxisListType.X, op=ALU.add)
                pr = wp.tile([2, 4], F32)
                nc.vector.tensor_copy(out=pr[:, 0:3], in_=th[:])
                nc.vector.tensor_scalar(out=pr[:, 3:4], in0=s[:], scalar1=127.5,
                                        scalar2=None, op0=ALU.add)
                nc.sync.dma_start(bass.AP(scr, b * 8, [[4, 2], [1, 4]]), pr[:])
                thb = wp.tile([P, 8], F32)
                nc.sync.dma_start(thb[:], bass.AP(scr, b * 8, [[0, P], [1, 8]]))
                # thb cols: 0=t00 1=t01 2=t02(unused) 3=cx 4=t10 5=t11 6=t12 7=cy

                # ---- build row-pair interleaved image copy in DRAM
                it = wp.tile([P, 1536], F32)
                nc.sync.dma_start(it[:], bass.AP(inp, b * (6 + IMG_ELS) + 6,
                                                 [[1536, P], [1, 1536]]))
                hal = wp.tile([P, ROW_ELS], F32)
                nc.sync.dma_start(hal[0:127, :],
                                  bass.AP(inp, b * (6 + IMG_ELS) + 6 + 1536,
                                          [[1536, 127], [1, ROW_ELS]]))
                nc.sync.dma_start(hal[127:128, :],
                                  bass.AP(inp, b * (6 + IMG_ELS) + 6 + IMG_ELS - ROW_ELS,
                                          [[ROW_ELS, 1], [1, ROW_ELS]]))
                d2 = wp.tile([P, PW, 6], F32)
                it3 = it[:].rearrange("p (w c) -> p w c", c=3)
                nc.vector.tensor_copy(out=d2[:, :, 0:3], in_=it3)
                nc.vector.tensor_copy(out=d2[:, 0:256, 3:6],
                                      in_=it[:, ROW_ELS:1536].rearrange("p (w c) -> p w c", c=3))
                nc.vector.tensor_copy(out=d2[:, 256:512, 3:6],
                                      in_=hal[:].rearrange("p (w c) -> p w c", c=3))
                nc.sync.dma_start(idups[b][:, :], d2[:])

                # ---- grid coords
                X = wp.tile([P, PW], F32)
                nc.vector.tensor_scalar(out=X[:], in0=xg[:], scalar1=thb[:, 0:1],
                                        scalar2=None, op0=ALU.mult)
                X2 = wp.tile([P, PW], F32)
                nc.vector.scalar_tensor_tensor(out=X2[:], in0=yg[:], scalar=thb[:, 1:2],
                                               in1=X[:], op0=ALU.mult, op1=ALU.add)
                nc.vector.tensor_scalar(out=X[:], in0=X2[:], scalar1=thb[:, 3:4],
                                        scalar2=None, op0=ALU.add)
                Y = wp.tile([P, PW], F32)
                nc.vector.tensor_scalar(out=Y[:], in0=xg[:], scalar1=thb[:, 4:5],
                                        scalar2=None, op0=ALU.mult)
                Y2 = wp.tile([P, PW], F32)
                nc.vector.scalar_tensor_tensor(out=Y2[:], in0=yg[:], scalar=thb[:, 5:6],
                                               in1=Y[:], op0=ALU.mult, op1=ALU.add)
                nc.vector.tensor_scalar(out=Y[:], in0=Y2[:], scalar1=thb[:, 7:8],
                                        scalar2=None, op0=ALU.add)

                # ---- floor via int truncation + correction
                def floor_of(src, nm):
                    ti = wp.tile([P, PW], I32, tag=f"fl_i{nm}")
                    nc.vector.tensor_copy(out=ti[:], in_=src[:])
                    tf = wp.tile([P, PW], F32, tag=f"fl_f{nm}")
                    nc.vector.tensor_copy(out=tf[:], in_=ti[:])
                    gt = wp.tile([P, PW], F32, tag=f"fl_g{nm}")
                    nc.vector.tensor_tensor(out=gt[:], in0=tf[:], in1=src[:], op=ALU.is_gt)
                    fl = wp.tile([P, PW], F32, tag=f"fl_o{nm}")
                    nc.vector.tensor_tensor(out=fl[:], in0=tf[:], in1=gt[:], op=ALU.subtract)
                    return fl

                xf = floor_of(X, "x")
                yf = floor_of(Y, "y")

                # ---- weights with OOB masks
                fx = wp.tile([P, PW], F32)
                nc.vector.tensor_tensor(out=fx[:], in0=X[:], in1=xf[:], op=ALU.subtract)
                fy = wp.tile([P, PW], F32)
                nc.vector.tensor_tensor(out=fy[:], in0=Y[:], in1=yf[:], op=ALU.subtract)
                al = wp.tile([P, PW], F32)
                nc.vector.tensor_scalar(out=al[:], in0=fx[:], scalar1=-1.0, scalar2=1.0,
                                        op0=ALU.mult, op1=ALU.add)
                ga = wp.tile([P, PW], F32)
                nc.vector.tensor_scalar(out=ga[:], in0=fy[:], scalar1=-1.0, scalar2=1.0,
                                        op0=ALU.mult, op1=ALU.add)
                mgx = wp.tile([P, PW], F32)
                nc.vector.tensor_scalar(out=mgx[:], in0=xf[:], scalar1=0.0, scalar2=None,
                                        op0=ALU.is_ge)
                mx = wp.tile([P, PW], F32)
                nc.vector.scalar_tensor_tensor(out=mx[:], in0=xf[:], scalar=254.0,
                                               in1=mgx[:], op0=ALU.is_le, op1=ALU.mult)
                mgy = wp.tile([P, PW], F32)
                nc.vector.tensor_scalar(out=mgy[:], in0=yf[:], scalar1=0.0, scalar2=None,
                                        op0=ALU.is_ge)
                my = wp.tile([P, PW], F32)
                nc.vector.scalar_tensor_tensor(out=my[:], in0=yf[:], scalar=254.0,
                                               in1=mgy[:], op0=ALU.is_le, op1=ALU.mult)
                A = wp.tile([P, PW], F32)
                nc.vector.tensor_tensor(out=A[:], in0=al[:], in1=mx[:], op=ALU.mult)
                Bw = wp.tile([P, PW], F32)
                nc.vector.tensor_tensor(out=Bw[:], in0=fx[:], in1=mx[:], op=ALU.mult)
                Cw = wp.tile([P, PW], F32)
                nc.vector.tensor_tensor(out=Cw[:], in0=ga[:], in1=my[:], op=ALU.mult)
                Dw = wp.tile([P, PW], F32)
                nc.vector.tensor_tensor(out=Dw[:], in0=fy[:], in1=my[:], op=ALU.mult)
                w00 = wp.tile([P, PW], F32)
                nc.vector.tensor_tensor(out=w00[:], in0=Cw[:], in1=A[:], op=ALU.mult)
                w10 = wp.tile([P, PW], F32)
                nc.vector.tensor_tensor(out=w10[:], in0=Dw[:], in1=A[:], op=ALU.mult)
                w01 = wp.tile([P, PW], F32)
                nc.vector.tensor_tensor(out=w01[:], in0=Cw[:], in1=Bw[:], op=ALU.mult)
                w11 = wp.tile([P, PW], F32)
                nc.vector.tensor_tensor(out=w11[:], in0=Dw[:], in1=Bw[:], op=ALU.mult)

                # ---- gather offsets: site = clamp(yf,0,254)*256 + clamp(xf,0,254)
                xc = wp.tile([P, PW], F32)
                nc.vector.tensor_scalar(out=xc[:], in0=xf[:], scalar1=0.0, scalar2=254.0,
                                        op0=ALU.max, op1=ALU.min)
                yc = wp.tile([P, PW], F32)
                nc.vector.tensor_scalar(out=yc[:], in0=yf[:], scalar1=0.0, scalar2=254.0,
                                        op0=ALU.max, op1=ALU.min)
                lf = wp.tile([P, PW], F32)
                nc.vector.scalar_tensor_tensor(out=lf[:], in0=yc[:], scalar=256.0,
                                               in1=xc[:], op0=ALU.mult, op1=ALU.add)
                off = opool.tile([P, PW], I32)
                nc.vector.tensor_copy(out=off[:], in_=lf[:])

                # ---- per-column [P,1] patch gathers, alternating SWDGE queues
                g = gpool.tile([P, PW, 12], F32)
                for w in range(PW):
                    inst = nc.gpsimd.indirect_dma_start(
                        out=g[:, w, :], out_offset=None,
                        in_=idups[b][:, :],
                        in_offset=bass.IndirectOffsetOnAxis(ap=off[:, w:w + 1], axis=0))
                    if w % 2:
                        inst.ins.queue = "qPoolDynamic1"

                # ---- blend: slices (r,s): 0:3=(0,0) 3:6=(1,0) 6:9=(0,1) 9:12=(1,1)
                def bc3(t):
                    return bass.AP(t.tensor, t.offset, list(t.ap) + [[0, 3]])

                t0 = wp.tile([P, PW, 3], F32)
                nc.vector.tensor_tensor(out=t0[:], in0=g[:, :, 0:3], in1=bc3(w00[:]), op=ALU.mult)
                t1 = wp.tile([P, PW, 3], F32)
                nc.vector.tensor_tensor(out=t1[:], in0=g[:, :, 3:6], in1=bc3(w10[:]), op=ALU.mult)
                t2 = wp.tile([P, PW, 3], F32)
                nc.vector.tensor_tensor(out=t2[:], in0=g[:, :, 6:9], in1=bc3(w01[:]), op=ALU.mult)
                t3 = wp.tile([P, PW, 3], F32)
                nc.vector.tensor_tensor(out=t3[:], in0=g[:, :, 9:12], in1=bc3(w11[:]), op=ALU.mult)
                nc.vector.tensor_tensor(out=t0[:], in0=t0[:], in1=t1[:], op=ALU.add)
                nc.vector.tensor_tensor(out=t2[:], in0=t2[:], in1=t3[:], op=ALU.add)
                ob = wp.tile([P, PW, 3], F32)
                nc.vector.tensor_tensor(out=ob[:], in0=t0[:], in1=t2[:], op=ALU.add)
                nc.sync.dma_start(bass.AP(out_d, b * IMG_ELS, [[PW * 3, P], [1, PW * 3]]),
                                  ob[:])
    nc.compile()
    return nc


def _consts():
    # pixel (p, w): global l = p*PW + w ; j = l % 256 ; i = l // 256
    l = np.arange(P * PW).reshape(P, PW)
    xg = (l % 256).astype(np.float32)
    yg = (l // 256).astype(np.float32)
    cst = np.tile(np.array([-127.5, -127.5, 127.5, 0.0], np.float32), (2, 1))
    return xg, yg, cst


IMGS_PER_LAUNCH = 16


def kernel(inputs: np.ndarray) -> np.ndarray:
    inputs = np.ascontiguousarray(inputs, dtype=np.float32)
    assert inputs.shape == (128, 6 + IMG_ELS)
    npl = IMGS_PER_LAUNCH
    if npl not in _cached:
        _cached[npl] = _build(npl)
    nc = _cached[npl]
    xg, yg, cst = _consts()
    trace = bool(os.environ.get("BILIN_TRACE"))
    if trace:
        try:  # NTFF trace hook is missing from this image's antenv; install shim
            import antenv.axon_hooks  # noqa: F401
        except ImportError:
            try:
                import types
                from trn_agent_boot.trn_boot import _ntff_profile_via_ctypes
                hook = _ntff_profile_via_ctypes("/opt/axon/libaxon_pjrt.so")
                mod = types.ModuleType("antenv.axon_hooks")
                mod.get_axon_ntff_profile_hook = lambda: hook
                sys.modules["antenv.axon_hooks"] = mod
            except Exception:
                trace = False
    out = np.empty((128, H, W, C), np.float32)
    total_ns = 0
    n_launches = IMGS // npl
    for k in range(n_launches):
        in_maps = []
        for c in range(N_CORES):
            lo = c * IMGS + k * npl
            in_maps.append(dict(inp=np.ascontiguousarray(inputs[lo:lo + npl]),
                                xg=xg, yg=yg, cst=cst))
        res = run_bass_kernel_spmd(nc, in_maps, core_ids=list(range(N_CORES)),
                                   trace=trace and k == 0)
        if trace and k == 0 and res.exec_time_ns is not None:
            total_ns = res.exec_time_ns * n_launches
        for c in range(N_CORES):
            lo = c * IMGS + k * npl
            out[lo:lo + npl] = res.results[c]["out"].reshape(npl, H, W, C)
    if trace:
        print(f"HW exec time: {total_ns} ns")
    return out

